# revision 13
# baseline (speedup 1.0000x reference)
"""CSA Lightning Indexer on 8 Trainium2 NeuronCores (Bass/Tile).

Reference computation (per batch b):
    qi = q[b] @ Wq.T            # [Lq, 2048] viewed as [Lq, H=4, Dh=512]
    ki = k[b] @ Wk.T            # [Lc, 2048] viewed as [Lc, 4, 512]
    w  = q[b] @ Wg.T            # [Lq, 4]
    scores[b,i,k] = sum_h relu(qi[i,h]·ki[k,h]) * w[i,h]

Sharding: (B=2, Lq=4096) flattened to 8192 query rows, 1024 rows per core
(cores 0-3 batch 0, cores 4-7 batch 1). The ki projection (shared by the 4
cores of a batch group) is sharded 4-way over its output dim: each core
computes 4 of the 16 j-tiles, the group AllGathers the full kiT through
DRAM bounce buffers while qproj runs, then every core reads the complete
[2048, 1024] kiT back into SBUF for the qk stage. This cuts per-core PE
work from ~298us to ~216us vs computing ki redundantly per core.

Device layout: all matmul contractions run over the SBUF partition dim, so
the host passes q/k transposed (d-major) and the weights pre-tiled into
[128, 16*128] column blocks (only the core's own 4 Wk blocks are sent).
Per-core pipeline:
  A) kiT_own[jl,kh] = sum_d WkT[d,j] kT[d,k]   (own quarter, -> DRAM ag_in)
     AllGather ag_in -> ag_out within each batch group of 4 cores
  B) qiT[j,i]  = sum_d WqT[d,j] qT[d,i]        (full i range, Wq streamed)
     wT[h,i]   = sum_d WgT[d,h] qT[d,i], then 4x128 PE transpose -> w[i,h]
     ag_out -> kiT SBUF tiles (all 16, uniform across cores)
  C) qk[i,k]   = sum_{j in h} qiT[j,i] kiT[j,k]   (PSUM, per head)
     scores[i,k] = sum_h relu(qk_h)*w_h: h=0 fused max*w on DVE; h>0
     Relu on the scalar(ACT) engine + fused (tm*w)+sc scalar_tensor_tensor
     on DVE, so GpSimd stays free and the tail after the last matmul is
     short. Output DMA per [128,512] chunk.

Matmuls run in float16 (11-bit significand, 2-byte dtype: fast weight load
+ half the DMA/SBUF/wire traffic) with fp32 PSUM accumulation; the tiny
gate-vector path stays float32r.
"""

import numpy as np

import concourse.bacc as bacc
import concourse.mybir as mybir
from concourse import tile
from concourse.bass_utils import run_bass_kernel_spmd

N_CORES = 8
B, LQ, LC, D, H, DH = 2, 4096, 1024, 2048, 4, 512
LQC = (B * LQ) // N_CORES  # 1024 query rows per core
ND = D // 128  # 16 d-tiles (contraction)
NJ = D // 128  # 16 j-tiles (projection output)
NJQ = NJ // 4  # 4 j-tiles computed locally for ki (quarter)
NKH = LC // 512  # 2 k halves
NIT = LQC // 128  # 8 i-tiles

F32 = mybir.dt.float32
R = mybir.dt.float16
RW = mybir.dt.float32r  # gate-vector path stays tf32-precision

REPLICA_GROUPS = [[0, 1, 2, 3], [4, 5, 6, 7]]

_CACHE = {}


def _build():
    nc = bacc.Bacc(num_devices=N_CORES)

    qT = nc.dram_tensor("qT", [D, LQC], R, kind="ExternalInput")
    kT = nc.dram_tensor("kT", [D, LC], R, kind="ExternalInput")
    wq2 = nc.dram_tensor("wq2", [NJ, 128, D], R, kind="ExternalInput")
    wk2q = nc.dram_tensor("wk2q", [NJQ, 128, D], R, kind="ExternalInput")
    wg2 = nc.dram_tensor("wg2", [128, ND * H], R, kind="ExternalInput")
    eye4 = nc.dram_tensor("eye4", [4, 4], RW, kind="ExternalInput")
    scores = nc.dram_tensor("scores", [LQC, LC], F32, kind="ExternalOutput")

    with tile.TileContext(nc) as tc:
        with (
            tc.tile_pool(name="kiT", bufs=1) as kiT_pool,
            tc.tile_pool(name="misc", bufs=1) as misc_pool,
            tc.tile_pool(name="dve", bufs=1) as dve_pool,
            tc.tile_pool(name="qT", bufs=1) as qT_pool,
            tc.tile_pool(name="wqblk", bufs=3) as wqblk_pool,
            tc.tile_pool(name="agdram", bufs=1, space="DRAM") as ag_pool,
        ):
            # chunk c = jl*2 + kh holds kiT j-tile (4*rank_in_group + jl),
            # k half kh; AllGather concatenates the 4 ranks' chunks so
            # ag_out[m*8 + jl*2 + kh] is global j-tile 4m+jl.
            ag_in = ag_pool.tile([NJQ * NKH, 128, 512], R, name="ag_in")
            ag_out = ag_pool.tile([4 * NJQ * NKH, 128, 512], R, name="ag_out")

            kiT = [
                kiT_pool.tile([128, LC], R, tag=f"kiT{j}", name=f"kiT{j}")
                for j in range(NJ)
            ]
            eye_sb = misc_pool.tile([4, 4], RW, tag="eye", name="eye_sb")
            nc.sync.dma_start(out=eye_sb[:], in_=eye4[:])
            wg_sb = misc_pool.tile([128, ND * H], R, tag="wg", name="wg_sb")
            nc.sync.dma_start(out=wg_sb[:], in_=wg2[:])

            # ------- stage A: own-quarter kiT = WkT.T-blocks @ kT --------
            with (
                nc.named_scope("kproj"),
                tc.tile_pool(name="kT", bufs=1) as kT_pool,
                tc.tile_pool(name="wkblk", bufs=2) as wkblk_pool,
                tc.tile_pool(name="kstg", bufs=8) as kstg_pool,
                tc.tile_pool(name="psA", bufs=4, space="PSUM") as psA_pool,
            ):
                # stage-A inputs (kT, Wk quarter) are issued FIRST: kproj
                # and the AllGather it feeds are the critical path; qT/wq
                # follow and land while kproj runs on the PE. Full-tile
                # transfers keep the per-partition runs at 2KB.
                wkb_sb = []
                for jl in range(NJQ):
                    wkb = wkblk_pool.tile([128, D], R, tag="wkb", name=f"wkb{jl}", bufs=4)
                    wkb_sb.append(wkb)
                nc.sync.dma_start(out=wkb_sb[0][:], in_=wk2q[0])
                kT_sb = []
                for dt in range(ND):
                    t = kT_pool.tile([128, LC], R, tag=f"kT{dt}", name=f"kT{dt}")
                    nc.sync.dma_start(out=t[:], in_=kT[dt * 128 : (dt + 1) * 128, :])
                    kT_sb.append(t)
                for jl in range(1, NJQ):
                    nc.sync.dma_start(out=wkb_sb[jl][:], in_=wk2q[jl])
                # stage-B inputs stream behind stage A's. The first 10 wq
                # blocks are prefetched so qproj never waits on the DMA
                # queues while the AllGather's SDMA drain is hogging them.
                qT_sb = []
                for dt in range(ND):
                    t = qT_pool.tile([128, LQC], R, tag=f"qT{dt}", name=f"qT{dt}")
                    nc.sync.dma_start(out=t[:], in_=qT[dt * 128 : (dt + 1) * 128, :])
                    qT_sb.append(t)
                wqb_head = []
                for jt in range(10):
                    wqb = wqblk_pool.tile([128, D], R, tag="wqb", name=f"wqb{jt}", bufs=10)
                    nc.sync.dma_start(out=wqb[:], in_=wq2[jt])
                    wqb_head.append(wqb)
                # kh0/kh1 chains run interleaved per jl so the PE issues two
                # matmuls per landing kT tile while the first tiles stream
                # in — the DMA ramp stays dense and HAM warms up on real
                # work (no dummy warmup needed).
                for jl in range(NJQ):
                    ps2 = [
                        psA_pool.tile([128, 512], F32, tag="psA", name=f"psA{jl}_{kh}")
                        for kh in range(NKH)
                    ]
                    for dt in range(ND):
                        for kh in range(NKH):
                            nc.tensor.matmul(
                                ps2[kh][:],
                                wkb_sb[jl][:, dt * 128 : (dt + 1) * 128],
                                kT_sb[dt][:, kh * 512 : (kh + 1) * 512],
                                start=(dt == 0),
                                stop=(dt == ND - 1),
                            )
                    for kh in range(NKH):
                        stg = kstg_pool.tile(
                            [128, 512], R, tag="kstg", name=f"kstg{jl}_{kh}"
                        )
                        nc.scalar.copy(stg[:], ps2[kh][:])
                        # on the sync queue the staging writes execute after
                        # the input bulk has drained — no serial ACT-queue
                        # bottleneck, and the collective machinery stays
                        # quiet during the input burst
                        nc.sync.dma_start(out=ag_in[jl * NKH + kh], in_=stg[:])

                nc.gpsimd.collective_compute(
                    "AllGather",
                    mybir.AluOpType.bypass,
                    replica_groups=REPLICA_GROUPS,
                    ins=[ag_in.opt()],
                    outs=[ag_out.opt()],
                )

            # ---------------- stage B: full i range ---------------------
            with (
                tc.tile_pool(name="qiT", bufs=1) as qiT_pool,
                tc.tile_pool(name="wsb", bufs=1) as w_pool,
                tc.tile_pool(name="sc", bufs=3) as sc_pool,
                tc.tile_pool(name="tm", bufs=4) as tm_pool,
            ):
                with (
                    tc.tile_pool(name="psB", bufs=2, space="PSUM") as psB_pool,
                    tc.tile_pool(name="psw", bufs=1, space="PSUM") as psw_pool,
                ):
                    # gate vector: wT[h, i] halves, then 4x128 PE transposes
                    with nc.named_scope("wproj"):
                        w4 = dve_pool.tile([4, LQC], RW, tag="w4", name="w4")
                        for ih in range(2):
                            psw = psw_pool.tile([4, 512], F32, tag="psw", name=f"psw{ih}")
                            for dt in range(ND):
                                nc.tensor.matmul(
                                    psw[:],
                                    wg_sb[:, dt * H : (dt + 1) * H],
                                    qT_sb[dt][:, ih * 512 : (ih + 1) * 512],
                                    start=(dt == 0),
                                    stop=(dt == ND - 1),
                                )
                            nc.vector.tensor_copy(w4[:, ih * 512 : (ih + 1) * 512], psw[:])
                        w_sb = []
                        for it in range(NIT):
                            pswt = psw_pool.tile([128, 4], F32, tag="pswt", name=f"pswt{it}")
                            nc.tensor.matmul(
                                pswt[:],
                                w4[:, it * 128 : (it + 1) * 128],
                                eye_sb[:],
                                start=True,
                                stop=True,
                            )
                            wt = w_pool.tile([128, 4], F32, tag=f"w{it}", name=f"w{it}")
                            nc.vector.tensor_copy(wt[:], pswt[:])
                            w_sb.append(wt)

                    # qiT = Wq-blocks.T @ qT (each weight block used once)
                    with nc.named_scope("qproj"):
                        qiT = []
                        for jt in range(NJ):
                            if jt < 10:
                                wqb = wqb_head[jt]
                            else:
                                wqb = wqblk_pool.tile([128, D], R, tag="wqb", name=f"wqb{jt}", bufs=10)
                                nc.sync.dma_start(out=wqb[:], in_=wq2[jt])
                            qi = qiT_pool.tile([128, LQC], R, tag=f"qiT{jt}", name=f"qiT{jt}")
                            for ih in range(2):
                                ps = psB_pool.tile([128, 512], F32, tag="psB", name=f"psB{jt}_{ih}")
                                for dt in range(ND):
                                    nc.tensor.matmul(
                                        ps[:],
                                        wqb[:, dt * 128 : (dt + 1) * 128],
                                        qT_sb[dt][:, ih * 512 : (ih + 1) * 512],
                                        start=(dt == 0),
                                        stop=(dt == ND - 1),
                                    )
                                nc.scalar.copy(qi[:, ih * 512 : (ih + 1) * 512], ps[:])
                            qiT.append(qi)

                    # full kiT comes back from the gather (uniform across
                    # cores: read all 16 tiles, own quarter included)
                    for m in range(4):
                        for jl in range(NJQ):
                            for kh in range(NKH):
                                nc.sync.dma_start(
                                    out=kiT[4 * m + jl][:, kh * 512 : (kh + 1) * 512],
                                    in_=ag_out[m * NJQ * NKH + jl * NKH + kh],
                                )

                # qk + fused relu*w epilogue
                with (
                    nc.named_scope("qk"),
                    tc.tile_pool(name="psq", bufs=6, space="PSUM") as psq_pool,
                ):
                    for it in range(NIT):
                        sc = sc_pool.tile([128, LC], F32, tag="sc", name=f"sc{it}")
                        for kh in range(NKH):
                            scs = sc[:, kh * 512 : (kh + 1) * 512]
                            for h in range(H):
                                psq = psq_pool.tile([128, 512], F32, tag="psq", name=f"psq{it}_{kh}_{h}")
                                for j in range(4):
                                    jt = h * 4 + j
                                    nc.tensor.matmul(
                                        psq[:],
                                        qiT[jt][:, it * 128 : (it + 1) * 128],
                                        kiT[jt][:, kh * 512 : (kh + 1) * 512],
                                        start=(j == 0),
                                        stop=(j == 3),
                                    )
                                if h == 0:
                                    nc.vector.tensor_scalar(
                                        out=scs,
                                        in0=psq[:],
                                        scalar1=0.0,
                                        scalar2=w_sb[it][:, 0:1],
                                        op0=mybir.AluOpType.max,
                                        op1=mybir.AluOpType.mult,
                                    )
                                else:
                                    tm = tm_pool.tile([128, 512], F32, tag="tm", name=f"tm{it}_{kh}_{h}")
                                    nc.scalar.activation(
                                        tm[:], psq[:], mybir.ActivationFunctionType.Relu
                                    )
                                    nc.vector.scalar_tensor_tensor(
                                        out=scs,
                                        in0=tm[:],
                                        scalar=w_sb[it][:, h : h + 1],
                                        in1=scs,
                                        op0=mybir.AluOpType.mult,
                                        op1=mybir.AluOpType.add,
                                    )
                            nc.sync.dma_start(
                                out=scores[
                                    it * 128 : (it + 1) * 128,
                                    kh * 512 : (kh + 1) * 512,
                                ],
                                in_=scs,
                            )
    nc.finalize()
    return nc


def _get_program():
    if "nc" not in _CACHE:
        _CACHE["nc"] = _build()
    return _CACHE["nc"]


def _tile_weight(w):
    # [j, d] nn.Linear weight -> [jt, p, dt*128+jcol] blocks where
    # block[jt][p, dt*128+j] = W.T[dt*128+p, jt*128+j]
    a = w.reshape(NJ, 128, ND, 128)  # [jt, j, dt, p]
    return np.ascontiguousarray(a.transpose(0, 3, 2, 1)).reshape(NJ, 128, D)


def _shard_inputs(q, k_compressed, Wq, Wk, Wg):
    ndt = np.float16
    wq2 = _tile_weight(np.asarray(Wq, dtype=np.float32)).astype(ndt)
    wk2 = _tile_weight(np.asarray(Wk, dtype=np.float32)).astype(ndt)
    # wg2[p, dt*4+h] = Wg.T[dt*128+p, h]
    g = np.asarray(Wg, dtype=np.float32).reshape(H, ND, 128)  # [h, dt, p]
    wg2 = np.ascontiguousarray(g.transpose(2, 1, 0)).reshape(128, ND * H).astype(ndt)
    eye = np.eye(4, dtype=np.float32)

    in_maps = []
    for c in range(N_CORES):
        b = c // (N_CORES // B)
        cq = c % (N_CORES // B)
        i0 = cq * LQC
        qTc = np.ascontiguousarray(
            np.asarray(q[b, i0 : i0 + LQC, :], dtype=np.float32).T
        ).astype(ndt)
        kTc = np.ascontiguousarray(
            np.asarray(k_compressed[b], dtype=np.float32).T
        ).astype(ndt)
        wk2q = np.ascontiguousarray(wk2[cq * NJQ : (cq + 1) * NJQ])
        in_maps.append(
            {"qT": qTc, "kT": kTc, "wq2": wq2, "wk2q": wk2q, "wg2": wg2, "eye4": eye}
        )
    return in_maps


def _run(inputs, trace=False, **kw):
    nc = _get_program()
    in_maps = _shard_inputs(**inputs)
    res = run_bass_kernel_spmd(nc, in_maps, list(range(N_CORES)), trace=trace, **kw)
    out = np.empty((B, LQ, LC), dtype=np.float32)
    for c in range(N_CORES):
        b = c // (N_CORES // B)
        i0 = (c % (N_CORES // B)) * LQC
        out[b, i0 : i0 + LQC, :] = res.results[c]["scores"]
    return out, res


def kernel(**inputs) -> np.ndarray:
    out, _ = _run(inputs)
    return out


# revision 14
# speedup vs baseline: 1.6186x; 1.6186x over previous
"""CSA Lightning Indexer on 8 Trainium2 NeuronCores (Bass/Tile).

Reference computation (per batch b):
    qi = q[b] @ Wq.T            # [Lq, 2048] viewed as [Lq, H=4, Dh=512]
    ki = k[b] @ Wk.T            # [Lc, 2048] viewed as [Lc, 4, 512]
    w  = q[b] @ Wg.T            # [Lq, 4]
    scores[b,i,k] = sum_h relu(qi[i,h]·ki[k,h]) * w[i,h]

Sharding: (B=2, Lq=4096) flattened to 8192 query rows, 1024 rows per core
(cores 0-3 batch 0, cores 4-7 batch 1). The ki projection (shared by the 4
cores of a batch group) is sharded 4-way over its output dim: each core
computes 4 of the 16 j-tiles, the group AllGathers the full kiT through
DRAM bounce buffers while qproj runs, then every core reads the complete
[2048, 1024] kiT back into SBUF for the qk stage. This cuts per-core PE
work from ~298us to ~216us vs computing ki redundantly per core.

Device layout: all matmul contractions run over the SBUF partition dim, so
the host passes q/k transposed (d-major) and the weights pre-tiled into
[128, 16*128] column blocks (only the core's own 4 Wk blocks are sent).
Per-core pipeline:
  A) kiT_own[jl,kh] = sum_d WkT[d,j] kT[d,k]   (own quarter, -> DRAM ag_in)
     AllGather ag_in -> ag_out within each batch group of 4 cores
  B) qiT[j,i]  = sum_d WqT[d,j] qT[d,i]        (full i range, Wq streamed)
     wT[h,i]   = sum_d WgT[d,h] qT[d,i], then 4x128 PE transpose -> w[i,h]
     ag_out -> kiT SBUF tiles (all 16, uniform across cores)
  C) qk[i,k]   = sum_{j in h} qiT[j,i] kiT[j,k]   (PSUM, per head)
     scores[i,k] = sum_h relu(qk_h)*w_h: h=0 fused max*w on DVE; h>0
     Relu on the scalar(ACT) engine + fused (tm*w)+sc scalar_tensor_tensor
     on DVE, so GpSimd stays free and the tail after the last matmul is
     short. Output DMA per [128,512] chunk.

Matmuls run in float16 (11-bit significand, 2-byte dtype: fast weight load
+ half the DMA/SBUF/wire traffic) with fp32 PSUM accumulation; the tiny
gate-vector path stays float32r.
"""

import numpy as np

import concourse.bacc as bacc
import concourse.mybir as mybir
from concourse import tile
from concourse.bass_utils import run_bass_kernel_spmd

N_CORES = 8
B, LQ, LC, D, H, DH = 2, 4096, 1024, 2048, 4, 512
LQC = (B * LQ) // N_CORES  # 1024 query rows per core
ND = D // 128  # 16 d-tiles (contraction)
NJ = D // 128  # 16 j-tiles (projection output)
NJQ = NJ // 4  # 4 j-tiles computed locally for ki (quarter)
NKH = LC // 512  # 2 k halves
NIT = LQC // 128  # 8 i-tiles

F32 = mybir.dt.float32
R = mybir.dt.float16
RW = mybir.dt.float32r  # gate-vector path stays tf32-precision

REPLICA_GROUPS = [[0, 1, 2, 3], [4, 5, 6, 7]]

_CACHE = {}


def _build():
    nc = bacc.Bacc(num_devices=N_CORES)

    qT = nc.dram_tensor("qT", [D, LQC], R, kind="ExternalInput")
    kT = nc.dram_tensor("kT", [D, LC], R, kind="ExternalInput")
    wq2 = nc.dram_tensor("wq2", [NJ, 128, D], R, kind="ExternalInput")
    wk2q = nc.dram_tensor("wk2q", [NJQ, 128, D], R, kind="ExternalInput")
    wg2 = nc.dram_tensor("wg2", [128, ND * H], R, kind="ExternalInput")
    eye4 = nc.dram_tensor("eye4", [4, 4], RW, kind="ExternalInput")
    scores = nc.dram_tensor("scores", [LQC, LC], F32, kind="ExternalOutput")

    with tile.TileContext(nc) as tc:
        with (
            tc.tile_pool(name="kiT", bufs=1) as kiT_pool,
            tc.tile_pool(name="misc", bufs=1) as misc_pool,
            tc.tile_pool(name="dve", bufs=1) as dve_pool,
            tc.tile_pool(name="qT", bufs=1) as qT_pool,
            tc.tile_pool(name="wqblk", bufs=3) as wqblk_pool,
            tc.tile_pool(name="agdram", bufs=1, space="DRAM") as ag_pool,
        ):
            # chunk c = jl*2 + kh holds kiT j-tile (4*rank_in_group + jl),
            # k half kh; AllGather concatenates the 4 ranks' chunks so
            # ag_out[m*8 + jl*2 + kh] is global j-tile 4m+jl.
            ag_in = ag_pool.tile([NJQ * NKH, 128, 512], R, name="ag_in")
            ag_out = ag_pool.tile([4 * NJQ * NKH, 128, 512], R, name="ag_out")

            kiT = [
                kiT_pool.tile([128, LC], R, tag=f"kiT{j}", name=f"kiT{j}")
                for j in range(NJ)
            ]
            eye_sb = misc_pool.tile([4, 4], RW, tag="eye", name="eye_sb")
            nc.sync.dma_start(out=eye_sb[:], in_=eye4[:])
            wg_sb = misc_pool.tile([128, ND * H], R, tag="wg", name="wg_sb")
            nc.sync.dma_start(out=wg_sb[:], in_=wg2[:])

            # ------- stage A: own-quarter kiT = WkT.T-blocks @ kT --------
            with (
                nc.named_scope("kproj"),
                tc.tile_pool(name="kT", bufs=1) as kT_pool,
                tc.tile_pool(name="wkblk", bufs=2) as wkblk_pool,
                tc.tile_pool(name="kstg", bufs=8) as kstg_pool,
                tc.tile_pool(name="psA", bufs=4, space="PSUM") as psA_pool,
            ):
                # stage-A inputs (kT, Wk quarter) are issued FIRST: kproj
                # and the AllGather it feeds are the critical path; qT/wq
                # follow and land while kproj runs on the PE. Full-tile
                # transfers keep the per-partition runs at 2KB.
                wkb_sb = []
                for jl in range(NJQ):
                    wkb = wkblk_pool.tile([128, D], R, tag="wkb", name=f"wkb{jl}", bufs=4)
                    wkb_sb.append(wkb)
                nc.sync.dma_start(out=wkb_sb[0][:], in_=wk2q[0])
                kT_sb = []
                for dt in range(ND):
                    t = kT_pool.tile([128, LC], R, tag=f"kT{dt}", name=f"kT{dt}")
                    nc.sync.dma_start(out=t[:], in_=kT[dt * 128 : (dt + 1) * 128, :])
                    kT_sb.append(t)
                for jl in range(1, NJQ):
                    nc.sync.dma_start(out=wkb_sb[jl][:], in_=wk2q[jl])
                # stage-B inputs stream behind stage A's. The first 10 wq
                # blocks are prefetched so qproj never waits on the DMA
                # queues while the AllGather's SDMA drain is hogging them.
                qT_sb = []
                for dt in range(ND):
                    t = qT_pool.tile([128, LQC], R, tag=f"qT{dt}", name=f"qT{dt}")
                    nc.sync.dma_start(out=t[:], in_=qT[dt * 128 : (dt + 1) * 128, :])
                    qT_sb.append(t)
                wqb_head = []
                for jt in range(10):
                    wqb = wqblk_pool.tile([128, D], R, tag="wqb", name=f"wqb{jt}", bufs=10)
                    nc.sync.dma_start(out=wqb[:], in_=wq2[jt])
                    wqb_head.append(wqb)
                # kh0/kh1 chains run interleaved per jl so the PE issues two
                # matmuls per landing kT tile while the first tiles stream
                # in — the DMA ramp stays dense and HAM warms up on real
                # work (no dummy warmup needed).
                for jl in range(NJQ):
                    ps2 = [
                        psA_pool.tile([128, 512], F32, tag="psA", name=f"psA{jl}_{kh}")
                        for kh in range(NKH)
                    ]
                    for dt in range(ND):
                        for kh in range(NKH):
                            nc.tensor.matmul(
                                ps2[kh][:],
                                wkb_sb[jl][:, dt * 128 : (dt + 1) * 128],
                                kT_sb[dt][:, kh * 512 : (kh + 1) * 512],
                                start=(dt == 0),
                                stop=(dt == ND - 1),
                            )
                    for kh in range(NKH):
                        stg = kstg_pool.tile(
                            [128, 512], R, tag="kstg", name=f"kstg{jl}_{kh}"
                        )
                        nc.scalar.copy(stg[:], ps2[kh][:])
                        # on the sync queue the staging writes execute after
                        # the input bulk has drained — no serial ACT-queue
                        # bottleneck, and the collective machinery stays
                        # quiet during the input burst
                        nc.sync.dma_start(out=ag_in[jl * NKH + kh], in_=stg[:])

                # The PE throttles to ~2.0GHz once the collective machinery
                # becomes active and never recovers within the kernel (any
                # CC, even 64B, triggers it; measured 216 -> 262 ns/MM).
                # Holding the CC dispatch behind the last wq prefetch keeps
                # the machinery quiet through the DMA-burst phase so the
                # early matmuls run at full clock.
                delay_t = misc_pool.tile([128, 8], R, tag="ccdel", name="ccdel")
                nc.gpsimd.tensor_copy(delay_t[:], wqb_head[9][:, 0:8])
                nc.gpsimd.collective_compute(
                    "AllGather",
                    mybir.AluOpType.bypass,
                    replica_groups=REPLICA_GROUPS,
                    ins=[ag_in.opt()],
                    outs=[ag_out.opt()],
                )

            # ---------------- stage B: full i range ---------------------
            with (
                tc.tile_pool(name="qiT", bufs=1) as qiT_pool,
                tc.tile_pool(name="wsb", bufs=1) as w_pool,
                tc.tile_pool(name="sc", bufs=3) as sc_pool,
                tc.tile_pool(name="tm", bufs=4) as tm_pool,
            ):
                with (
                    tc.tile_pool(name="psB", bufs=2, space="PSUM") as psB_pool,
                    tc.tile_pool(name="psw", bufs=1, space="PSUM") as psw_pool,
                ):
                    # gate vector: wT[h, i] halves, then 4x128 PE transposes
                    with nc.named_scope("wproj"):
                        w4 = dve_pool.tile([4, LQC], RW, tag="w4", name="w4")
                        for ih in range(2):
                            psw = psw_pool.tile([4, 512], F32, tag="psw", name=f"psw{ih}")
                            for dt in range(ND):
                                nc.tensor.matmul(
                                    psw[:],
                                    wg_sb[:, dt * H : (dt + 1) * H],
                                    qT_sb[dt][:, ih * 512 : (ih + 1) * 512],
                                    start=(dt == 0),
                                    stop=(dt == ND - 1),
                                )
                            nc.vector.tensor_copy(w4[:, ih * 512 : (ih + 1) * 512], psw[:])
                        w_sb = []
                        for it in range(NIT):
                            pswt = psw_pool.tile([128, 4], F32, tag="pswt", name=f"pswt{it}")
                            nc.tensor.matmul(
                                pswt[:],
                                w4[:, it * 128 : (it + 1) * 128],
                                eye_sb[:],
                                start=True,
                                stop=True,
                            )
                            wt = w_pool.tile([128, 4], F32, tag=f"w{it}", name=f"w{it}")
                            nc.vector.tensor_copy(wt[:], pswt[:])
                            w_sb.append(wt)

                    # qiT = Wq-blocks.T @ qT (each weight block used once)
                    with nc.named_scope("qproj"):
                        qiT = []
                        for jt in range(NJ):
                            if jt < 10:
                                wqb = wqb_head[jt]
                            else:
                                wqb = wqblk_pool.tile([128, D], R, tag="wqb", name=f"wqb{jt}", bufs=10)
                                nc.sync.dma_start(out=wqb[:], in_=wq2[jt])
                            qi = qiT_pool.tile([128, LQC], R, tag=f"qiT{jt}", name=f"qiT{jt}")
                            for ih in range(2):
                                ps = psB_pool.tile([128, 512], F32, tag="psB", name=f"psB{jt}_{ih}")
                                for dt in range(ND):
                                    nc.tensor.matmul(
                                        ps[:],
                                        wqb[:, dt * 128 : (dt + 1) * 128],
                                        qT_sb[dt][:, ih * 512 : (ih + 1) * 512],
                                        start=(dt == 0),
                                        stop=(dt == ND - 1),
                                    )
                                nc.scalar.copy(qi[:, ih * 512 : (ih + 1) * 512], ps[:])
                            qiT.append(qi)

                    # full kiT comes back from the gather (uniform across
                    # cores: read all 16 tiles, own quarter included)
                    for m in range(4):
                        for jl in range(NJQ):
                            for kh in range(NKH):
                                nc.sync.dma_start(
                                    out=kiT[4 * m + jl][:, kh * 512 : (kh + 1) * 512],
                                    in_=ag_out[m * NJQ * NKH + jl * NKH + kh],
                                )

                # qk + fused relu*w epilogue
                with (
                    nc.named_scope("qk"),
                    tc.tile_pool(name="psq", bufs=6, space="PSUM") as psq_pool,
                ):
                    for it in range(NIT):
                        sc = sc_pool.tile([128, LC], F32, tag="sc", name=f"sc{it}")
                        for kh in range(NKH):
                            scs = sc[:, kh * 512 : (kh + 1) * 512]
                            for h in range(H):
                                psq = psq_pool.tile([128, 512], F32, tag="psq", name=f"psq{it}_{kh}_{h}")
                                for j in range(4):
                                    jt = h * 4 + j
                                    nc.tensor.matmul(
                                        psq[:],
                                        qiT[jt][:, it * 128 : (it + 1) * 128],
                                        kiT[jt][:, kh * 512 : (kh + 1) * 512],
                                        start=(j == 0),
                                        stop=(j == 3),
                                    )
                                if h == 0:
                                    nc.vector.tensor_scalar(
                                        out=scs,
                                        in0=psq[:],
                                        scalar1=0.0,
                                        scalar2=w_sb[it][:, 0:1],
                                        op0=mybir.AluOpType.max,
                                        op1=mybir.AluOpType.mult,
                                    )
                                else:
                                    tm = tm_pool.tile([128, 512], F32, tag="tm", name=f"tm{it}_{kh}_{h}")
                                    nc.scalar.activation(
                                        tm[:], psq[:], mybir.ActivationFunctionType.Relu
                                    )
                                    nc.vector.scalar_tensor_tensor(
                                        out=scs,
                                        in0=tm[:],
                                        scalar=w_sb[it][:, h : h + 1],
                                        in1=scs,
                                        op0=mybir.AluOpType.mult,
                                        op1=mybir.AluOpType.add,
                                    )
                            nc.sync.dma_start(
                                out=scores[
                                    it * 128 : (it + 1) * 128,
                                    kh * 512 : (kh + 1) * 512,
                                ],
                                in_=scs,
                            )
    nc.finalize()
    return nc


def _get_program():
    if "nc" not in _CACHE:
        _CACHE["nc"] = _build()
    return _CACHE["nc"]


def _tile_weight(w):
    # [j, d] nn.Linear weight -> [jt, p, dt*128+jcol] blocks where
    # block[jt][p, dt*128+j] = W.T[dt*128+p, jt*128+j]
    a = w.reshape(NJ, 128, ND, 128)  # [jt, j, dt, p]
    return np.ascontiguousarray(a.transpose(0, 3, 2, 1)).reshape(NJ, 128, D)


def _shard_inputs(q, k_compressed, Wq, Wk, Wg):
    ndt = np.float16
    wq2 = _tile_weight(np.asarray(Wq, dtype=np.float32)).astype(ndt)
    wk2 = _tile_weight(np.asarray(Wk, dtype=np.float32)).astype(ndt)
    # wg2[p, dt*4+h] = Wg.T[dt*128+p, h]
    g = np.asarray(Wg, dtype=np.float32).reshape(H, ND, 128)  # [h, dt, p]
    wg2 = np.ascontiguousarray(g.transpose(2, 1, 0)).reshape(128, ND * H).astype(ndt)
    eye = np.eye(4, dtype=np.float32)

    in_maps = []
    for c in range(N_CORES):
        b = c // (N_CORES // B)
        cq = c % (N_CORES // B)
        i0 = cq * LQC
        qTc = np.ascontiguousarray(
            np.asarray(q[b, i0 : i0 + LQC, :], dtype=np.float32).T
        ).astype(ndt)
        kTc = np.ascontiguousarray(
            np.asarray(k_compressed[b], dtype=np.float32).T
        ).astype(ndt)
        wk2q = np.ascontiguousarray(wk2[cq * NJQ : (cq + 1) * NJQ])
        in_maps.append(
            {"qT": qTc, "kT": kTc, "wq2": wq2, "wk2q": wk2q, "wg2": wg2, "eye4": eye}
        )
    return in_maps


def _run(inputs, trace=False, **kw):
    nc = _get_program()
    in_maps = _shard_inputs(**inputs)
    res = run_bass_kernel_spmd(nc, in_maps, list(range(N_CORES)), trace=trace, **kw)
    out = np.empty((B, LQ, LC), dtype=np.float32)
    for c in range(N_CORES):
        b = c // (N_CORES // B)
        i0 = (c % (N_CORES // B)) * LQC
        out[b, i0 : i0 + LQC, :] = res.results[c]["scores"]
    return out, res


def kernel(**inputs) -> np.ndarray:
    out, _ = _run(inputs)
    return out
